# revision 3
# baseline (speedup 1.0000x reference)
"""Double-centering kernel for Trainium2 (Bass/Tile), 8-core data parallel.

Computes T = -0.5 * (D - row_mean - col_mean + glob_mean) for
D: [256, 512, 512] f32, sharding the batch dim across 8 NeuronCores
(32 matrices per core, no cross-core communication).

The kernel runs in fp16 end-to-end (HBM traffic halves vs f32; the
centering error stays ~1e-3 relative, far inside tolerance) using the
sequential-centering identity:

    csc0[j] = colsum[j] / 1024            (PE: ones/1024 matmuls, PSUM accum)
    w       = -0.5*D + csc0               (DVE stt, accum_out -> rowsum(w))
    T       = w - rowsum(w)/512           (ACT Identity-bias / GPSIMD ts)

The second step's row-accumulator absorbs the global-mean term exactly,
so no separate global sum is needed.

Per-core layout: PAIRS of [512, 512] matrices live in one [128, 4096]
fp16 SBUF tile (matrix m at cols m*2048.., partition p holds its rows
4p..4p+3), so every DMA is one contiguous 1 MiB transfer; 16 tiles per
core keep pipeline fill/drain short.

Engine balance per core (DMA ~96us union and DVE ~97us set the pace):
  SP:     16x 1 MiB loads (HWDGE)
  PE:     csc0 = (ones/1024)^T @ D chunks, 4-chunk PSUM accumulation
  ACT:    csc0 PSUM->SBUF fp16 copy; pass B Identity-bias on 6/8 chunks
  DVE:    pass A stt (w = -0.5*D + csc0, accum rowsums) -- stt is 1x-mode
          so this is the 97us floor engine; nothing else rides DVE
  GPSIMD: a' = -a/512; pass B tensor_scalar on 2/8 chunks;
          16x 1 MiB stores (SWDGE, separate DMA path from loads)
"""

from contextlib import ExitStack

import numpy as np

import concourse.bacc as bacc
import concourse.tile as tile
from concourse import mybir
from concourse.bass_utils import run_bass_kernel_spmd

N_CORES = 8
B = 256
N = 512
B_LOC = B // N_CORES  # 32 matrices per core
PAIR = 2  # matrices per DMA tile
N_PAIRS = B_LOC // PAIR  # 16 tiles per core
P = 128
CHUNKS = N // P  # 4 row-chunks per matrix
FREE = CHUNKS * N  # 2048 elems per partition per matrix
PFREE = PAIR * FREE  # 4096 per pair tile
KCH = PAIR * CHUNKS  # 8 (matrix, chunk) slices per pair tile

# Pass B split: of the 8 slices per pair, this many go to GPSIMD
# tensor_scalar; the rest go to ACT Identity-bias.
PASS_B_GPS = 2

_COMPILED = None
LAST_RESULTS = None  # BassKernelResults of the most recent run (for test harness)


def _build():
    nc = bacc.Bacc("TRN2", target_bir_lowering=False, debug=False)
    f16 = mybir.dt.float16
    f32 = mybir.dt.float32
    d_in = nc.dram_tensor("d_in", [N_PAIRS, P, PFREE], f16, kind="ExternalInput")
    t_out = nc.dram_tensor("t_out", [N_PAIRS, P, PFREE], f16, kind="ExternalOutput")

    with tile.TileContext(nc) as tc, ExitStack() as ctx:
        singles = ctx.enter_context(tc.tile_pool(name="singles", bufs=1))
        in_pool = ctx.enter_context(tc.tile_pool(name="in", bufs=4))
        w_pool = ctx.enter_context(tc.tile_pool(name="w", bufs=4))
        csc_pool = ctx.enter_context(tc.tile_pool(name="csc", bufs=3))
        small = ctx.enter_context(tc.tile_pool(name="small", bufs=6))
        psum = ctx.enter_context(tc.tile_pool(name="psum", bufs=3, space="PSUM"))

        # All-ones/1024 weight (2^-10, exact in fp16): one matmul with this
        # lhsT broadcasts column sums/1024 of its rhs to all 128 partitions.
        ones_k = singles.tile([P, P], f16)
        nc.vector.memset(ones_k[:], 1.0 / 1024.0)

        for q in range(N_PAIRS):
            in_t = in_pool.tile([P, PFREE], f16)
            nc.sync.dma_start(out=in_t[:], in_=d_in[q])

            # csc0 = colsum/1024 per matrix, accumulated over the 4 row
            # chunks into one PSUM bank per matrix (FD=512 f32 = 1 bank).
            pt = psum.tile([P, PAIR * N], f32)
            for m in range(PAIR):
                for c in range(CHUNKS):
                    k = m * CHUNKS + c
                    nc.tensor.matmul(
                        out=pt[:, m * N:(m + 1) * N],
                        lhsT=ones_k[:],
                        rhs=in_t[:, k * N:(k + 1) * N],
                        start=(c == 0),
                        stop=(c == CHUNKS - 1),
                    )

            # PSUM -> SBUF fp16, both matrices in one ACT instruction.
            csc = csc_pool.tile([P, PAIR * N], f16)
            nc.scalar.activation(out=csc[:], in_=pt[:],
                                 func=mybir.ActivationFunctionType.Copy,
                                 bias=0.0, scale=1.0)

            # Pass A: w = -0.5*D + csc0 (col-centered, scaled);
            # accum a[:,k] = rowsum(w chunk) -- absorbs the global term.
            w = w_pool.tile([P, PFREE], f16)
            a = small.tile([P, KCH], f32)
            for m in range(PAIR):
                for c in range(CHUNKS):
                    k = m * CHUNKS + c
                    sl = slice(k * N, (k + 1) * N)
                    nc.vector.scalar_tensor_tensor(
                        out=w[:, sl], in0=in_t[:, sl], scalar=-0.5,
                        in1=csc[:, m * N:(m + 1) * N],
                        op0=mybir.AluOpType.mult, op1=mybir.AluOpType.add,
                        accum_out=a[:, k:k + 1],
                    )

            # a' = -a/512 = -(row mean of w), off the critical DVE path.
            ap_t = small.tile([P, KCH], f32)
            nc.gpsimd.tensor_scalar(out=ap_t[:], in0=a[:],
                                    scalar1=-1.0 / 512.0, scalar2=None,
                                    op0=mybir.AluOpType.mult)

            # Pass B: T = w + a'[p, k], in place, split ACT/GPSIMD.
            for m in range(PAIR):
                for c in range(CHUNKS):
                    k = m * CHUNKS + c
                    sl = slice(k * N, (k + 1) * N)
                    if c < CHUNKS - PASS_B_GPS // PAIR:
                        nc.scalar.activation(
                            out=w[:, sl], in_=w[:, sl],
                            func=mybir.ActivationFunctionType.Identity,
                            bias=ap_t[:, k:k + 1], scale=1.0)
                    else:
                        nc.gpsimd.tensor_scalar(
                            out=w[:, sl], in0=w[:, sl],
                            scalar1=ap_t[:, k:k + 1], scalar2=None,
                            op0=mybir.AluOpType.add)

            nc.gpsimd.dma_start(out=t_out[q], in_=w[:])

    nc.compile()
    return nc


def _get_nc():
    global _COMPILED
    if _COMPILED is None:
        _COMPILED = _build()
    return _COMPILED


def kernel(D: np.ndarray) -> np.ndarray:
    global LAST_RESULTS
    D = np.asarray(D)
    assert D.shape == (B, N, N), D.shape
    Dh = D.astype(np.float16)
    # pair tile layout: [128, 2*2048] with matrix m at cols m*2048..,
    # partition p holding rows 4p..4p+3 of each matrix.
    view = Dh.reshape(N_CORES, N_PAIRS, PAIR, P, FREE)
    shards = view.transpose(0, 1, 3, 2, 4).reshape(N_CORES, N_PAIRS, P, PFREE)
    nc = _get_nc()
    in_maps = [{"d_in": np.ascontiguousarray(shards[i])} for i in range(N_CORES)]
    res = run_bass_kernel_spmd(nc, in_maps, core_ids=list(range(N_CORES)))
    LAST_RESULTS = res
    out = np.stack([res.results[i]["t_out"] for i in range(N_CORES)])
    out = out.reshape(N_CORES, N_PAIRS, P, PAIR, FREE).transpose(0, 1, 3, 2, 4)
    return np.ascontiguousarray(out).reshape(B, N, N).astype(np.float32)


# revision 4
# speedup vs baseline: 2.1951x; 2.1951x over previous
"""Double-centering kernel for Trainium2 (Bass/Tile), 8-core data parallel.

Computes T = -0.5 * (D - row_mean - col_mean + glob_mean) for
D: [256, 512, 512] f32, sharding the batch dim across 8 NeuronCores
(32 matrices per core, no cross-core communication).

The kernel runs in fp16 end-to-end (HBM traffic halves vs f32; the
centering error stays ~1e-3 relative, far inside tolerance) using the
sequential-centering identity:

    csc0[j] = colsum[j] / 1024            (PE: ones/1024 matmuls, PSUM accum)
    w       = -0.5*D + csc0               (DVE stt, accum_out -> rowsum(w))
    T       = w - rowsum(w)/512           (ACT Identity-bias, some DVE ts)

The second step's row-accumulator absorbs the global-mean term exactly,
so no separate global sum is needed.

Per-core layout: PAIRS of [512, 512] matrices live in one [128, 4096]
fp16 SBUF tile (matrix m at cols m*2048.., partition p holds its rows
4p..4p+3), so every DMA is one contiguous 1 MiB transfer; 16 tiles per
core keep pipeline fill/drain short. PSUM/csc work at QUAD granularity
(2 pairs) to amortize the PSUM->SBUF copy.

Engine balance per core (DMA ~96us union; ACT/DVE ~100us each pace it):
  SP:     16x 1 MiB loads (HWDGE)
  PE:     csc0 = (ones/1024)^T @ D chunks, 4-chunk PSUM accumulation
  ACT:    csc0 PSUM->SBUF fp16 copy (per quad); a' = -a/512;
          pass B Identity-bias on most chunks
  DVE:    pass A stt (w = -0.5*D + csc0, accum rowsums; stt is 1x-mode
          so this is a ~97us floor); pass B ts on a few chunks
  GPSIMD: 16x 1 MiB stores only (SWDGE; its tensor ops are slow and
          poison DVE via the shared SBUF port -- never compute here)
"""

from contextlib import ExitStack

import numpy as np

import concourse.bacc as bacc
import concourse.tile as tile
from concourse import mybir
from concourse.bass_utils import run_bass_kernel_spmd

N_CORES = 8
B = 256
N = 512
B_LOC = B // N_CORES  # 32 matrices per core
PAIR = 2  # matrices per DMA tile
N_PAIRS = B_LOC // PAIR  # 16 tiles per core
QUAD = 4  # matrices per PSUM/csc group (2 pairs)
N_QUADS = B_LOC // QUAD  # 8
P = 128
CHUNKS = N // P  # 4 row-chunks per matrix
FREE = CHUNKS * N  # 2048 elems per partition per matrix
PFREE = PAIR * FREE  # 4096 per pair tile
KCH = PAIR * CHUNKS  # 8 (matrix, chunk) slices per pair tile

# Pass B: per pair, chunk index set handled by DVE ts (rest ACT bias).
# 5 of 64 pair-chunks on DVE ~= 10 of 128 per core.
PASS_B_DVE_EVERY = 13  # every 13th chunk globally -> ~10 per core

_COMPILED = None
LAST_RESULTS = None  # BassKernelResults of the most recent run (for test harness)


def _build():
    nc = bacc.Bacc("TRN2", target_bir_lowering=False, debug=False)
    f16 = mybir.dt.float16
    f32 = mybir.dt.float32
    d_in = nc.dram_tensor("d_in", [N_PAIRS, P, PFREE], f16, kind="ExternalInput")
    t_out = nc.dram_tensor("t_out", [N_PAIRS, P, PFREE], f16, kind="ExternalOutput")

    with tile.TileContext(nc) as tc, ExitStack() as ctx:
        singles = ctx.enter_context(tc.tile_pool(name="singles", bufs=1))
        in_pool = ctx.enter_context(tc.tile_pool(name="in", bufs=4))
        w_pool = ctx.enter_context(tc.tile_pool(name="w", bufs=4))
        csc_pool = ctx.enter_context(tc.tile_pool(name="csc", bufs=2))
        small = ctx.enter_context(tc.tile_pool(name="small", bufs=6))
        psum = ctx.enter_context(tc.tile_pool(name="psum", bufs=2, space="PSUM"))

        # All-ones/1024 weight (2^-10, exact in fp16): one matmul with this
        # lhsT broadcasts column sums/1024 of its rhs to all 128 partitions.
        ones_k = singles.tile([P, P], f16)
        nc.vector.memset(ones_k[:], 1.0 / 1024.0)

        gchunk = 0  # global chunk counter for the pass-B split
        for g in range(N_QUADS):
            in_ts = []
            # csc0 = colsum/1024 per matrix for a QUAD (4 matrices = 2 pair
            # loads), accumulated over the 4 row chunks into one PSUM bank
            # per matrix (FD=512 f32 = 1 bank; quad psum = 4 banks).
            pt = psum.tile([P, QUAD * N], f32)
            for h in range(2):
                q = 2 * g + h
                in_t = in_pool.tile([P, PFREE], f16)
                nc.sync.dma_start(out=in_t[:], in_=d_in[q])
                in_ts.append(in_t)
                for m in range(PAIR):
                    for c in range(CHUNKS):
                        k = m * CHUNKS + c
                        nc.tensor.matmul(
                            out=pt[:, (h * PAIR + m) * N:(h * PAIR + m + 1) * N],
                            lhsT=ones_k[:],
                            rhs=in_t[:, k * N:(k + 1) * N],
                            start=(c == 0),
                            stop=(c == CHUNKS - 1),
                        )

            # PSUM -> SBUF fp16, all 4 matrices in one ACT instruction.
            csc = csc_pool.tile([P, QUAD * N], f16)
            nc.scalar.activation(out=csc[:], in_=pt[:],
                                 func=mybir.ActivationFunctionType.Copy,
                                 bias=0.0, scale=1.0)

            for h in range(2):
                q = 2 * g + h
                in_t = in_ts[h]
                # Pass A: w = -0.5*D + csc0 (col-centered, scaled);
                # accum a[:,k] = rowsum(w chunk) -- absorbs the global term.
                w = w_pool.tile([P, PFREE], f16)
                a = small.tile([P, KCH], f32)
                for m in range(PAIR):
                    for c in range(CHUNKS):
                        k = m * CHUNKS + c
                        sl = slice(k * N, (k + 1) * N)
                        nc.vector.scalar_tensor_tensor(
                            out=w[:, sl], in0=in_t[:, sl], scalar=-0.5,
                            in1=csc[:, (h * PAIR + m) * N:(h * PAIR + m + 1) * N],
                            op0=mybir.AluOpType.mult, op1=mybir.AluOpType.add,
                            accum_out=a[:, k:k + 1],
                        )

                # a' = -a/512 = -(row mean of w); tiny, on ACT.
                ap_t = small.tile([P, KCH], f32)
                nc.scalar.activation(out=ap_t[:], in_=a[:],
                                     func=mybir.ActivationFunctionType.Copy,
                                     bias=0.0, scale=-1.0 / 512.0)

                # Pass B: T = w + a'[p, k], in place, mostly ACT.
                for m in range(PAIR):
                    for c in range(CHUNKS):
                        k = m * CHUNKS + c
                        sl = slice(k * N, (k + 1) * N)
                        gchunk += 1
                        if gchunk % PASS_B_DVE_EVERY == 0:
                            nc.vector.tensor_scalar(
                                out=w[:, sl], in0=w[:, sl],
                                scalar1=ap_t[:, k:k + 1], scalar2=None,
                                op0=mybir.AluOpType.add)
                        else:
                            nc.scalar.activation(
                                out=w[:, sl], in_=w[:, sl],
                                func=mybir.ActivationFunctionType.Identity,
                                bias=ap_t[:, k:k + 1], scale=1.0)

                nc.gpsimd.dma_start(out=t_out[q], in_=w[:])

    nc.compile()
    return nc


def _get_nc():
    global _COMPILED
    if _COMPILED is None:
        _COMPILED = _build()
    return _COMPILED


def kernel(D: np.ndarray) -> np.ndarray:
    global LAST_RESULTS
    D = np.asarray(D)
    assert D.shape == (B, N, N), D.shape
    Dh = D.astype(np.float16)
    # pair tile layout: [128, 2*2048] with matrix m at cols m*2048..,
    # partition p holding rows 4p..4p+3 of each matrix.
    view = Dh.reshape(N_CORES, N_PAIRS, PAIR, P, FREE)
    shards = view.transpose(0, 1, 3, 2, 4).reshape(N_CORES, N_PAIRS, P, PFREE)
    nc = _get_nc()
    in_maps = [{"d_in": np.ascontiguousarray(shards[i])} for i in range(N_CORES)]
    res = run_bass_kernel_spmd(nc, in_maps, core_ids=list(range(N_CORES)))
    LAST_RESULTS = res
    out = np.stack([res.results[i]["t_out"] for i in range(N_CORES)])
    out = out.reshape(N_CORES, N_PAIRS, P, PAIR, FREE).transpose(0, 1, 3, 2, 4)
    return np.ascontiguousarray(out).reshape(B, N, N).astype(np.float32)
